# revision 23
# baseline (speedup 1.0000x reference)
"""Trainium2 Bass kernel for nn_LogicalGNNLayer (GNN message passing + MLP).

Computation (reference):
    h = term_emb[heads]; t = term_emb[tails]           # gather  [E,B,D]
    agg = segsum(s*(h+pred), tails) + segsum(s*(t+inv), heads)   # [T,B,D]
    agg += EPS*term_emb
    out = relu(agg @ W1 + b1) @ W2 + b2                # [T,B,D]

Strategy:
  - Shard batch B across 8 cores (data parallel, Bc=512 per core); the
    term/edge structure and MLP weights are replicated.
  - The gather/scatter structure depends only on the tiny heads/tails index
    arrays: read them on the host and bake the message structure into the
    kernel as a static program.
  - On-chip layout is transposed: d on partitions, (t, b) on the free axis,
    so the MLP matmuls (which contract D) consume the aggregation output
    directly with no on-device transposes.
  - The PE is the bottleneck engine and is issue-rate-bound at ~1 moving
    column/cycle @2.4GHz (measured: ~216ns per 512-col matmul regardless of
    dtype/perf-mode; fp8 DoubleRow buys K=256-per-column packing, not
    column rate, and removing per-pass LDWEIGHTS does not change the rate).
    Per-rep PE floor = 68 segsum passes + 256 MLP passes = ~70us.
  - Aggregation per destination term k (per 128-partition d-tile):
      * the emb slices destined for k are DMA'd as one contiguous tile and
        segment-summed on the PE as fp8 DoubleRow identity matmuls into
        PSUM (PE is immune to the DMA/SBUF contention that inflates
        DVE/GpSimd op cost),
      * acc[k] = EPS*term[k] + psum segsum in one scalar_tensor_tensor,
      * + one add per distinct (dst,src) term edge (coefficients merged).
    Units are split between DVE and GpSimd by a cost-model LPT greedy
    using clean DVE rates on purpose: G-heavy assignments balance engine
    totals but lengthen the per-chunk dependency chains that pace the PE,
    which measures worse (re-confirmed: measured-contention rates gave
    82.2us vs 76.1us).
  - The 4MB term tile is filled by four range DMAs (k-quarters) so evicts
    and adds wait only on the quarter they read, not the whole transfer.
  - MLP: fp16 matmuls on PE with fp32 PSUM accumulation, software-pipelined
    in chunks of 2 term slots (psum: 2 tags x 2 bufs x 2 banks = 8 banks);
    ReLU and the output epilogue run on the scalar engine out of PSUM.
  - fp16 on-chip and fp16 output DMA (host upcasts) halve HBM traffic;
    emb tiles are fp8 (measured rel err vs the fp32 reference ~1.35e-2,
    within the 2e-2 gate; no further precision reduction fits the budget —
    fp8 anywhere else measures >2e-2 in numpy simulation).
"""

import numpy as np

import concourse.bass as bass
import concourse.tile as tile
from concourse import bacc, mybir
from concourse.bass_utils import run_bass_kernel_spmd

T, B, D, H, E = 16, 4096, 256, 512, 32
EPS = 0.1
N_CORES = 8
BC = B // N_CORES            # 512 batch per core
NB = T * BC                  # 8192 free-axis span (t, b)
DT = D // 128                # 2 d-tiles
HT = H // 128                # 4 h-tiles
NMSG = 2 * E                 # 64 directed messages
F16 = mybir.dt.float16
F32 = mybir.dt.float32
F8 = mybir.dt.float8e4

_KERNEL_CACHE = {}


def _messages(heads, tails, signs):
    """Directed message list (dst, src, sign, which_emb, e), sorted by dst."""
    msgs = []
    for e in range(E):
        h, t, s = int(heads[e]), int(tails[e]), float(signs[e])
        assert 0 <= h < T and 0 <= t < T
        msgs.append((t, h, s, 0, e))   # msg_to_tail: acc[t] += s*(term[h]+pred[e])
        msgs.append((h, t, s, 1, e))   # msg_to_head: acc[h] += s*(term[t]+inv[e])
    msgs.sort(key=lambda m: m[0])
    return msgs


def _plan(msgs):
    """Static schedule: emb groups, merged term edges,
    DVE/GpSimd assignment per (k, dt) unit."""
    slots = [[] for _ in range(T)]
    for m, (dst, _src, _s, _w, _e) in enumerate(msgs):
        slots[dst].append(m)
    gspan = []
    for k in range(T):
        if slots[k]:
            m0, g = slots[k][0], len(slots[k])
            assert slots[k] == list(range(m0, m0 + g))
            gspan.append((m0, g))
        else:
            gspan.append((0, 0))

    termops = []
    for k in range(T):
        c = {}
        for dst, src, s, _w, _e in msgs:
            if dst == k:
                c[src] = c.get(src, 0.0) + s
        termops.append([("term", src, v)
                        for src, v in sorted(c.items()) if v != 0.0])

    tmps = []

    # Engine split: the emb segment-sum runs on PE (identity matmuls into
    # PSUM, immune to DMA/SBUF contention); the DVE evicts psum with a fused
    # EPS*term init (scalar_tensor_tensor, DVE-only, PSUM reads don't
    # contend); term adds balance between DVE and GpSimd. Clean-rate DVE
    # numbers on purpose: G-heavy assignments balance engine totals but
    # lengthen the per-chunk dependency chains that pace the PE, which
    # measures worse.
    V_TT, G_TT, V_EVICT = 678.0, 2247.0, 1192.0
    units = list(range(T))
    assign = {}
    tv = tg = 0.0
    ucost = {k: len(termops[k]) for k in units}
    for u in sorted(units, key=lambda u: -ucost[u]):
        n = ucost[u]
        m_v = max(tv + V_EVICT + n * V_TT, tg)
        m_g = max(tv + V_EVICT, tg + n * G_TT)
        if n == 0 or m_v <= m_g:
            assign[u] = "v"
            tv += V_EVICT + n * V_TT
        else:
            assign[u] = "g"
            tv += V_EVICT
            tg += n * G_TT
    gmax = max(1, max(g for _m0, g in gspan))
    return gspan, termops, tmps, assign, gmax


def _build(msgs_key, repeats=1, loop=0, bias_zero=(True, True)):
    """Build + compile the per-core SPMD Bass program for a message structure.

    repeats: statically unroll the whole body N times (timing).
    loop: wrap the body in an on-device For_i loop of N iterations (timing).
    bias_zero: (b1 is all-zero, b2 is all-zero) — picks cheaper epilogues.
    """
    key = (msgs_key, repeats, loop, bias_zero)
    if key in _KERNEL_CACHE:
        return _KERNEL_CACHE[key]
    msgs = list(msgs_key)
    AF = mybir.ActivationFunctionType
    OP = mybir.AluOpType
    gspan, termops, tmps, assign, gmax = _plan(msgs)
    b1_zero, b2_zero = bias_zero

    nc = bacc.Bacc("TRN2", target_bir_lowering=False, debug=False,
                   num_devices=N_CORES)
    # termT layout: [p=128, (k, dt, b)] — both d-tiles of a term slot are
    # column-adjacent so evicts/term-adds cover them in one wide op.
    termT = nc.declare_dram_parameter("termT", [128, T * DT * BC], F16,
                                      isOutput=False)
    embT = nc.declare_dram_parameter("embT", [D, NMSG, BC], F8, isOutput=False)
    w1d = nc.declare_dram_parameter("w1", [D, H], F16, isOutput=False)
    w2d = nc.declare_dram_parameter("w2", [H, D], F16, isOutput=False)
    b1d = nc.declare_dram_parameter("b1t", [128, HT], F32, isOutput=False)
    b2d = nc.declare_dram_parameter("b2t", [128, DT], F32, isOutput=False)
    identd = nc.declare_dram_parameter("ident", [128, 2, 128], F8,
                                       isOutput=False)
    outT = nc.declare_dram_parameter("outT", [D, NB], F16, isOutput=True)

    with nc.allow_low_precision(reason="fp16 on-chip aggregation"), \
            tile.TileContext(nc) as tc, \
            tc.tile_pool(name="const", bufs=1) as cpool, \
            tc.tile_pool(name="term", bufs=2) as tpool, \
            tc.tile_pool(name="acc", bufs=1) as apool, \
            tc.tile_pool(name="emb", bufs=10) as epool, \
            tc.tile_pool(name="hid", bufs=3) as hpool, \
            tc.tile_pool(name="out", bufs=6) as opool, \
            tc.tile_pool(name="psagg", bufs=1, space="PSUM") as paggpool, \
            tc.tile_pool(name="psmlp", bufs=2, space="PSUM") as pspool:

        # ---- persistent loads -------------------------------------------
        w1s = []
        w2s = []
        for dt in range(DT):
            w = cpool.tile([128, H], F16, tag=f"w1_{dt}")
            nc.sync.dma_start(w[:], w1d[dt * 128:(dt + 1) * 128, :])
            w1s.append(w)
        for ht in range(HT):
            w = cpool.tile([128, D], F16, tag=f"w2_{ht}")
            nc.sync.dma_start(w[:], w2d[ht * 128:(ht + 1) * 128, :])
            w2s.append(w)
        b1s = cpool.tile([128, HT], F32, tag="b1")
        nc.sync.dma_start(b1s[:], b1d[:])
        b2s = cpool.tile([128, DT], F32, tag="b2")
        nc.sync.dma_start(b2s[:], b2d[:])
        ident = cpool.tile([128, 2, 128], F8, tag="ident")
        nc.sync.dma_start(ident[:], identd[:])

        carry = {"hids": None}

        def body(final=True):
            terms = [None]
            accs = [None] * T
            paggs = {}
            hids = {}

            def emit_emb_dma(c):
                tiles = []
                for ki in range(2):
                    k = 2 * c + ki
                    m0, g = gspan[k]
                    if not g:
                        tiles.append(None)
                        continue
                    per_dt = []
                    for dt in range(DT):
                        et = epool.tile([128, gmax, BC], F8, tag="emb")
                        nc.sync.dma_start(
                            et[:, :g, :],
                            embT[dt * 128:(dt + 1) * 128, m0:m0 + g, :])
                        per_dt.append(et)
                    tiles.append(per_dt)
                return tiles

            def emit_embmm(c, pre=None):
                # Segment-sum of the (fp8) emb slices for term slots 2c, 2c+1
                # as identity matmuls accumulating in PSUM: PE is immune to
                # the DMA/SBUF contention that triples DVE/GpSimd op cost,
                # and fp8 DoubleRow sums two slices per 512-row pass.
                DR = mybir.MatmulPerfMode.DoubleRow
                tiles = pre if pre is not None else emit_emb_dma(c)
                for ki in range(2):
                    k = 2 * c + ki
                    g = gspan[k][1]
                    if not g:
                        continue
                    pagg = paggpool.tile([128, 1024], F32, tag=f"pagg_{ki}")
                    paggs[k] = pagg
                    # dt-halves interleave so consecutive DR passes hit
                    # alternating PSUM banks: back-to-back accumulation
                    # into the same bank issues at ~400ns instead of 216
                    # (read-modify-write hazard); alternation hides it.
                    # Accumulation-group order within each bank is intact.
                    for j in range(0, g - 1, 2):
                        for dt in range(DT):
                            nc.tensor.matmul(
                                pagg[:, dt * 512:(dt + 1) * 512],
                                ident[:], tiles[ki][dt][:, j:j + 2, :],
                                perf_mode=DR,
                                start=(j == 0), stop=(j + 2 == g))
                    if g % 2:
                        for dt in range(DT):
                            nc.tensor.matmul(
                                pagg[:, dt * 512:(dt + 1) * 512],
                                ident[:, 0, :],
                                tiles[ki][dt][:, g - 1:g, :],
                                start=(g == 1), stop=True)

            def emit_evict(k):
                # evict: acc = EPS*term[k] + psum segsum, one wide op over
                # both d-tiles (DVE only — TensorScalarPtr is unsupported on
                # Pool, and PSUM reads don't contend with DMA SBUF writes).
                g = gspan[k][1]
                acc = apool.tile([128, 2 * BC], F16, tag=f"acc_{k}")
                accs[k] = acc
                tk = terms[0][:, k * 2 * BC:(k + 1) * 2 * BC]
                if g:
                    nc.vector.scalar_tensor_tensor(
                        acc[:], tk, EPS, paggs[k][:], OP.mult, OP.add)
                else:
                    nc.vector.tensor_scalar_mul(acc[:], tk, EPS)

            def emit_adds(k):
                # term-edge adds on the assigned engine
                eng = nc.vector if assign[k] == "v" else nc.gpsimd
                acc = accs[k]
                for kind, idx, c in termops[k]:
                    ts = terms[0][:, idx * 2 * BC:(idx + 1) * 2 * BC]
                    if c == 1.0:
                        eng.tensor_add(acc[:], acc[:], ts)
                    elif c == -1.0:
                        eng.tensor_sub(acc[:], acc[:], ts)
                    else:
                        nc.vector.scalar_tensor_tensor(acc[:], ts, c, acc[:],
                                                       OP.mult, OP.add)

            def emit_l1(c):
                for ht in range(HT):
                    ps = pspool.tile([128, 1024], F32, tag="mlp")
                    for dt in range(DT):
                        w = w1s[dt][:, ht * 128:(ht + 1) * 128]
                        for ki in range(2):
                            k = 2 * c + ki
                            nc.tensor.matmul(
                                ps[:, ki * 512:(ki + 1) * 512], w,
                                accs[k][:, dt * 512:(dt + 1) * 512],
                                start=(dt == 0), stop=(dt == DT - 1))
                    hid = hpool.tile([128, 1024], F16, tag=f"hid_{ht}")
                    if b1_zero:
                        nc.scalar.activation(hid[:], ps[:], AF.Relu,
                                             bias=0.0, scale=1.0)
                    else:
                        nc.scalar.activation(hid[:], ps[:], AF.Relu,
                                             bias=b1s[:, ht:ht + 1], scale=1.0)
                    hids[(c, ht)] = hid

            def emit_l2(c, hidmap=None):
                hidmap = hidmap if hidmap is not None else hids
                for dt2 in range(DT):
                    ps2 = pspool.tile([128, 1024], F32, tag="mlp")
                    for ht in range(HT):
                        w = w2s[ht][:, dt2 * 128:(dt2 + 1) * 128]
                        for ki in range(2):
                            nc.tensor.matmul(
                                ps2[:, ki * 512:(ki + 1) * 512], w,
                                hidmap[(c, ht)][:, ki * 512:(ki + 1) * 512],
                                start=(ht == 0), stop=(ht == HT - 1))
                    ot = opool.tile([128, 1024], F16, tag="ot")
                    if b2_zero:
                        nc.scalar.activation(ot[:], ps2[:], AF.Copy,
                                             bias=0.0, scale=1.0)
                    else:
                        nc.scalar.activation(ot[:], ps2[:], AF.Identity,
                                             bias=b2s[:, dt2:dt2 + 1],
                                             scale=1.0)
                    nc.sync.dma_start(
                        outT[dt2 * 128:(dt2 + 1) * 128,
                             2 * c * BC:(2 * c + 2) * BC], ot[:])

            # emb DMAs for the first two chunks go ahead of the term DMA so
            # their data lands while the carried L2 runs; the term tile is
            # filled by four k-quarter range DMAs so early evicts/adds wait
            # only on the quarter they actually read. The carried L2 runs
            # on the PE BEFORE the chunk-0/1 seg matmuls: it needs only
            # hid(7) (ready as the previous rep drains) and buys the
            # previous rep's chunk-7 evict time to free the pagg banks that
            # seg_mm(0) reuses (the recurring ~1.6us seam gap).
            pre0 = emit_emb_dma(0)
            pre1 = emit_emb_dma(1)
            tt = tpool.tile([128, T * DT * BC], F16, tag="term")
            TQ = T * DT * BC // 4
            for qi in range(4):
                nc.sync.dma_start(tt[:, qi * TQ:(qi + 1) * TQ],
                                  termT[:, qi * TQ:(qi + 1) * TQ])
            terms[0] = tt
            if carry["hids"] is not None:
                emit_l2(T // 2 - 1, carry["hids"])
                carry["hids"] = None
            emit_embmm(0, pre=pre0)
            emit_embmm(1, pre=pre1)
            for c in range(T // 2):
                k0, k1 = 2 * c, 2 * c + 1
                # both evicts first, always: evict(k1) must not queue behind
                # adds(k0) on the in-order DVE — it frees the pagg PSUM bank
                # that gates the chunk-(c+1) seg matmuls (and, at chunk 7,
                # the next rep's seg_mm(0)), and it unblocks a gpsimd
                # partner's adds early
                emit_evict(k0)
                emit_evict(k1)
                emit_adds(k0)
                emit_adds(k1)
                if c + 2 < T // 2:
                    emit_embmm(c + 2)
                if c > 0:
                    emit_l2(c - 1)
                emit_l1(c)
            if final:
                emit_l2(T // 2 - 1)
            else:
                carry["hids"] = hids

        if loop:
            ET = mybir.EngineType
            with tc.For_i(0, loop, 1,
                          hint_engines=(ET.PE, ET.DVE, ET.Activation, ET.SP)):
                body()
        else:
            for rep in range(repeats):
                body(final=(rep == repeats - 1))

    nc.compile()
    _KERNEL_CACHE[key] = nc
    return nc


def _prep_inputs(term_emb, pred_emb, inv_pred_emb, W1, b1, W2, b2, msgs):
    """Shard/transpose/cast host-side into the per-core device layouts."""
    import ml_dtypes
    f8 = ml_dtypes.float8_e4m3
    t16 = term_emb.astype(np.float16)
    emb = np.empty((NMSG, B, D), f8)
    for m, (_dst, _src, s, which, e) in enumerate(msgs):
        arr = pred_emb if which == 0 else inv_pred_emb
        if s == 1.0:
            emb[m] = arr[e].astype(f8)
        else:
            emb[m] = (s * arr[e]).astype(f8)
    w1_16 = np.ascontiguousarray(W1.astype(np.float16))
    w2_16 = np.ascontiguousarray(W2.astype(np.float16))
    b1t = np.ascontiguousarray(b1.astype(np.float32).reshape(HT, 128).T)
    b2t = np.ascontiguousarray(b2.astype(np.float32).reshape(DT, 128).T)
    ident = np.broadcast_to(np.eye(128, dtype=f8)[:, None, :],
                            (128, 2, 128))
    ident = np.ascontiguousarray(ident)
    in_maps = []
    for c in range(N_CORES):
        sl = slice(c * BC, (c + 1) * BC)
        termTc = np.ascontiguousarray(
            t16[:, sl, :].transpose(2, 0, 1).reshape(DT, 128, T, BC)
            .transpose(1, 2, 0, 3)).reshape(128, T * DT * BC)
        embTc = np.ascontiguousarray(
            emb[:, sl, :].transpose(2, 0, 1)).reshape(D, NMSG, BC)
        in_maps.append(dict(termT=termTc, embT=embTc, w1=w1_16, w2=w2_16,
                            b1t=b1t, b2t=b2t, ident=ident))
    return in_maps


def kernel(term_emb, pred_emb, inv_pred_emb, signs, W1, b1, W2, b2,
           heads, tails):
    term_emb = np.asarray(term_emb, dtype=np.float32)
    pred_emb = np.asarray(pred_emb, dtype=np.float32)
    inv_pred_emb = np.asarray(inv_pred_emb, dtype=np.float32)
    signs = np.asarray(signs, dtype=np.float32)
    W1 = np.asarray(W1, dtype=np.float32)
    b1 = np.asarray(b1, dtype=np.float32)
    W2 = np.asarray(W2, dtype=np.float32)
    b2 = np.asarray(b2, dtype=np.float32)
    heads = np.asarray(heads).astype(np.int64)
    tails = np.asarray(tails).astype(np.int64)

    msgs = _messages(heads, tails, signs)
    bias_zero = (not b1.any(), not b2.any())
    nc = _build(tuple(msgs), bias_zero=bias_zero)
    in_maps = _prep_inputs(term_emb, pred_emb, inv_pred_emb, W1, b1, W2, b2,
                           msgs)
    res = run_bass_kernel_spmd(nc, in_maps, list(range(N_CORES)))

    out = np.empty((T, B, D), np.float32)
    for c in range(N_CORES):
        o = res.results[c]["outT"].astype(np.float32)
        out[:, c * BC:(c + 1) * BC, :] = o.reshape(D, T, BC).transpose(1, 2, 0)
    return out


# revision 25
# speedup vs baseline: 1.0023x; 1.0023x over previous
"""Trainium2 Bass kernel for nn_LogicalGNNLayer (GNN message passing + MLP).

Computation (reference):
    h = term_emb[heads]; t = term_emb[tails]           # gather  [E,B,D]
    agg = segsum(s*(h+pred), tails) + segsum(s*(t+inv), heads)   # [T,B,D]
    agg += EPS*term_emb
    out = relu(agg @ W1 + b1) @ W2 + b2                # [T,B,D]

Strategy:
  - Shard batch B across 8 cores (data parallel, Bc=512 per core); the
    term/edge structure and MLP weights are replicated.
  - The gather/scatter structure depends only on the tiny heads/tails index
    arrays: read them on the host and bake the message structure into the
    kernel as a static program.
  - On-chip layout is transposed: d on partitions, (t, b) on the free axis,
    so the MLP matmuls (which contract D) consume the aggregation output
    directly with no on-device transposes.
  - The PE is the bottleneck engine and is issue-rate-bound at ~1 moving
    column/cycle @2.4GHz (measured: ~216ns per 512-col matmul regardless of
    dtype/perf-mode; fp8 DoubleRow buys K=256-per-column packing, not
    column rate, and removing per-pass LDWEIGHTS does not change the rate).
    Per-rep PE floor = 68 segsum passes + 256 MLP passes = ~70us.
  - Aggregation per destination term k (per 128-partition d-tile):
      * the emb slices destined for k are DMA'd as one contiguous tile and
        segment-summed on the PE as fp8 DoubleRow identity matmuls into
        PSUM (PE is immune to the DMA/SBUF contention that inflates
        DVE/GpSimd op cost),
      * acc[k] = EPS*term[k] + psum segsum in one scalar_tensor_tensor,
      * + one add per distinct (dst,src) term edge (coefficients merged).
    Units are split between DVE and GpSimd by a cost-model LPT greedy
    using clean DVE rates on purpose: G-heavy assignments balance engine
    totals but lengthen the per-chunk dependency chains that pace the PE,
    which measures worse (re-confirmed: measured-contention rates gave
    82.2us vs 76.1us).
  - The 4MB term tile is filled by four range DMAs (k-quarters) so evicts
    and adds wait only on the quarter they read, not the whole transfer.
  - Per chunk both evicts are emitted before any adds (an evict queued
    behind a long add chain on the in-order DVE delays freeing the pagg
    PSUM banks that gate the seg matmuls two chunks later); across the
    rep seam the carried last-chunk L2 runs on the PE before the chunk-0/1
    seg matmuls (whose emb DMAs still issue first), buying the previous
    rep's chunk-7 evict time to free the PSUM banks seg_mm(0) reuses.
  - MLP: fp16 matmuls on PE with fp32 PSUM accumulation, software-pipelined
    in chunks of 2 term slots (psum: 2 tags x 2 bufs x 2 banks = 8 banks);
    ReLU and the output epilogue run on the scalar engine out of PSUM.
  - fp16 on-chip and fp16 output DMA (host upcasts) halve HBM traffic;
    emb tiles are fp8 (measured rel err vs the fp32 reference ~1.35e-2,
    within the 2e-2 gate; no further precision reduction fits the budget —
    fp8 anywhere else measures >2e-2 in numpy simulation).
"""

import numpy as np

import concourse.bass as bass
import concourse.tile as tile
from concourse import bacc, mybir
from concourse.bass_utils import run_bass_kernel_spmd

T, B, D, H, E = 16, 4096, 256, 512, 32
EPS = 0.1
N_CORES = 8
BC = B // N_CORES            # 512 batch per core
NB = T * BC                  # 8192 free-axis span (t, b)
DT = D // 128                # 2 d-tiles
HT = H // 128                # 4 h-tiles
NMSG = 2 * E                 # 64 directed messages
F16 = mybir.dt.float16
F32 = mybir.dt.float32
F8 = mybir.dt.float8e4

_KERNEL_CACHE = {}


def _messages(heads, tails, signs):
    """Directed message list (dst, src, sign, which_emb, e), sorted by dst."""
    msgs = []
    for e in range(E):
        h, t, s = int(heads[e]), int(tails[e]), float(signs[e])
        assert 0 <= h < T and 0 <= t < T
        msgs.append((t, h, s, 0, e))   # msg_to_tail: acc[t] += s*(term[h]+pred[e])
        msgs.append((h, t, s, 1, e))   # msg_to_head: acc[h] += s*(term[t]+inv[e])
    msgs.sort(key=lambda m: m[0])
    return msgs


def _plan(msgs):
    """Static schedule: emb groups, merged term edges,
    DVE/GpSimd assignment per (k, dt) unit."""
    slots = [[] for _ in range(T)]
    for m, (dst, _src, _s, _w, _e) in enumerate(msgs):
        slots[dst].append(m)
    gspan = []
    for k in range(T):
        if slots[k]:
            m0, g = slots[k][0], len(slots[k])
            assert slots[k] == list(range(m0, m0 + g))
            gspan.append((m0, g))
        else:
            gspan.append((0, 0))

    termops = []
    for k in range(T):
        c = {}
        for dst, src, s, _w, _e in msgs:
            if dst == k:
                c[src] = c.get(src, 0.0) + s
        termops.append([("term", src, v)
                        for src, v in sorted(c.items()) if v != 0.0])

    tmps = []

    # Engine split: the emb segment-sum runs on PE (identity matmuls into
    # PSUM, immune to DMA/SBUF contention); the DVE evicts psum with a fused
    # EPS*term init (scalar_tensor_tensor, DVE-only, PSUM reads don't
    # contend); term adds balance between DVE and GpSimd. Clean-rate DVE
    # numbers on purpose: G-heavy assignments balance engine totals but
    # lengthen the per-chunk dependency chains that pace the PE, which
    # measures worse.
    V_TT, G_TT, V_EVICT = 678.0, 2247.0, 1192.0
    units = list(range(T))
    assign = {}
    tv = tg = 0.0
    ucost = {k: len(termops[k]) for k in units}
    for u in sorted(units, key=lambda u: -ucost[u]):
        n = ucost[u]
        m_v = max(tv + V_EVICT + n * V_TT, tg)
        m_g = max(tv + V_EVICT, tg + n * G_TT)
        if n == 0 or m_v <= m_g:
            assign[u] = "v"
            tv += V_EVICT + n * V_TT
        else:
            assign[u] = "g"
            tv += V_EVICT
            tg += n * G_TT
    gmax = max(1, max(g for _m0, g in gspan))
    return gspan, termops, tmps, assign, gmax


def _build(msgs_key, repeats=1, loop=0, bias_zero=(True, True)):
    """Build + compile the per-core SPMD Bass program for a message structure.

    repeats: statically unroll the whole body N times (timing).
    loop: wrap the body in an on-device For_i loop of N iterations (timing).
    bias_zero: (b1 is all-zero, b2 is all-zero) — picks cheaper epilogues.
    """
    key = (msgs_key, repeats, loop, bias_zero)
    if key in _KERNEL_CACHE:
        return _KERNEL_CACHE[key]
    msgs = list(msgs_key)
    AF = mybir.ActivationFunctionType
    OP = mybir.AluOpType
    gspan, termops, tmps, assign, gmax = _plan(msgs)
    b1_zero, b2_zero = bias_zero

    nc = bacc.Bacc("TRN2", target_bir_lowering=False, debug=False,
                   num_devices=N_CORES)
    # termT layout: [p=128, (k, dt, b)] — both d-tiles of a term slot are
    # column-adjacent so evicts/term-adds cover them in one wide op.
    termT = nc.declare_dram_parameter("termT", [128, T * DT * BC], F16,
                                      isOutput=False)
    embT = nc.declare_dram_parameter("embT", [D, NMSG, BC], F8, isOutput=False)
    w1d = nc.declare_dram_parameter("w1", [D, H], F16, isOutput=False)
    w2d = nc.declare_dram_parameter("w2", [H, D], F16, isOutput=False)
    b1d = nc.declare_dram_parameter("b1t", [128, HT], F32, isOutput=False)
    b2d = nc.declare_dram_parameter("b2t", [128, DT], F32, isOutput=False)
    identd = nc.declare_dram_parameter("ident", [128, 2, 128], F8,
                                       isOutput=False)
    outT = nc.declare_dram_parameter("outT", [D, NB], F16, isOutput=True)

    with nc.allow_low_precision(reason="fp16 on-chip aggregation"), \
            tile.TileContext(nc) as tc, \
            tc.tile_pool(name="const", bufs=1) as cpool, \
            tc.tile_pool(name="term", bufs=2) as tpool, \
            tc.tile_pool(name="acc", bufs=1) as apool, \
            tc.tile_pool(name="emb", bufs=10) as epool, \
            tc.tile_pool(name="hid", bufs=3) as hpool, \
            tc.tile_pool(name="out", bufs=6) as opool, \
            tc.tile_pool(name="psagg", bufs=1, space="PSUM") as paggpool, \
            tc.tile_pool(name="psmlp", bufs=2, space="PSUM") as pspool:

        # ---- persistent loads -------------------------------------------
        w1s = []
        w2s = []
        for dt in range(DT):
            w = cpool.tile([128, H], F16, tag=f"w1_{dt}")
            nc.sync.dma_start(w[:], w1d[dt * 128:(dt + 1) * 128, :])
            w1s.append(w)
        for ht in range(HT):
            w = cpool.tile([128, D], F16, tag=f"w2_{ht}")
            nc.sync.dma_start(w[:], w2d[ht * 128:(ht + 1) * 128, :])
            w2s.append(w)
        b1s = cpool.tile([128, HT], F32, tag="b1")
        nc.sync.dma_start(b1s[:], b1d[:])
        b2s = cpool.tile([128, DT], F32, tag="b2")
        nc.sync.dma_start(b2s[:], b2d[:])
        ident = cpool.tile([128, 2, 128], F8, tag="ident")
        nc.sync.dma_start(ident[:], identd[:])

        carry = {"hids": None}

        def body(final=True):
            terms = [None]
            accs = [None] * T
            paggs = {}
            hids = {}

            def emit_emb_dma(c):
                tiles = []
                for ki in range(2):
                    k = 2 * c + ki
                    m0, g = gspan[k]
                    if not g:
                        tiles.append(None)
                        continue
                    per_dt = []
                    for dt in range(DT):
                        et = epool.tile([128, gmax, BC], F8, tag="emb")
                        nc.sync.dma_start(
                            et[:, :g, :],
                            embT[dt * 128:(dt + 1) * 128, m0:m0 + g, :])
                        per_dt.append(et)
                    tiles.append(per_dt)
                return tiles

            def emit_embmm(c, pre=None):
                # Segment-sum of the (fp8) emb slices for term slots 2c, 2c+1
                # as identity matmuls accumulating in PSUM: PE is immune to
                # the DMA/SBUF contention that triples DVE/GpSimd op cost,
                # and fp8 DoubleRow sums two slices per 512-row pass.
                DR = mybir.MatmulPerfMode.DoubleRow
                tiles = pre if pre is not None else emit_emb_dma(c)
                for ki in range(2):
                    k = 2 * c + ki
                    g = gspan[k][1]
                    if not g:
                        continue
                    pagg = paggpool.tile([128, 1024], F32, tag=f"pagg_{ki}")
                    paggs[k] = pagg
                    # dt-major emission measured equal-or-better than
                    # interleaving the two PSUM halves (75164 vs 75394)
                    for dt in range(DT):
                        et = tiles[ki][dt]
                        half = pagg[:, dt * 512:(dt + 1) * 512]
                        for j in range(0, g - 1, 2):
                            nc.tensor.matmul(
                                half, ident[:], et[:, j:j + 2, :],
                                perf_mode=DR,
                                start=(j == 0), stop=(j + 2 == g))
                        if g % 2:
                            nc.tensor.matmul(
                                half, ident[:, 0, :], et[:, g - 1:g, :],
                                start=(g == 1), stop=True)

            def emit_evict(k):
                # evict: acc = EPS*term[k] + psum segsum, one wide op over
                # both d-tiles (DVE only — TensorScalarPtr is unsupported on
                # Pool, and PSUM reads don't contend with DMA SBUF writes).
                g = gspan[k][1]
                acc = apool.tile([128, 2 * BC], F16, tag=f"acc_{k}")
                accs[k] = acc
                tk = terms[0][:, k * 2 * BC:(k + 1) * 2 * BC]
                if g:
                    nc.vector.scalar_tensor_tensor(
                        acc[:], tk, EPS, paggs[k][:], OP.mult, OP.add)
                else:
                    nc.vector.tensor_scalar_mul(acc[:], tk, EPS)

            def emit_adds(k):
                # term-edge adds on the assigned engine
                eng = nc.vector if assign[k] == "v" else nc.gpsimd
                acc = accs[k]
                for kind, idx, c in termops[k]:
                    ts = terms[0][:, idx * 2 * BC:(idx + 1) * 2 * BC]
                    if c == 1.0:
                        eng.tensor_add(acc[:], acc[:], ts)
                    elif c == -1.0:
                        eng.tensor_sub(acc[:], acc[:], ts)
                    else:
                        nc.vector.scalar_tensor_tensor(acc[:], ts, c, acc[:],
                                                       OP.mult, OP.add)

            def emit_l1(c):
                for ht in range(HT):
                    ps = pspool.tile([128, 1024], F32, tag="mlp")
                    for dt in range(DT):
                        w = w1s[dt][:, ht * 128:(ht + 1) * 128]
                        for ki in range(2):
                            k = 2 * c + ki
                            nc.tensor.matmul(
                                ps[:, ki * 512:(ki + 1) * 512], w,
                                accs[k][:, dt * 512:(dt + 1) * 512],
                                start=(dt == 0), stop=(dt == DT - 1))
                    hid = hpool.tile([128, 1024], F16, tag=f"hid_{ht}")
                    if b1_zero:
                        nc.scalar.activation(hid[:], ps[:], AF.Relu,
                                             bias=0.0, scale=1.0)
                    else:
                        nc.scalar.activation(hid[:], ps[:], AF.Relu,
                                             bias=b1s[:, ht:ht + 1], scale=1.0)
                    hids[(c, ht)] = hid

            def emit_l2(c, hidmap=None):
                hidmap = hidmap if hidmap is not None else hids
                for dt2 in range(DT):
                    ps2 = pspool.tile([128, 1024], F32, tag="mlp")
                    for ht in range(HT):
                        w = w2s[ht][:, dt2 * 128:(dt2 + 1) * 128]
                        for ki in range(2):
                            nc.tensor.matmul(
                                ps2[:, ki * 512:(ki + 1) * 512], w,
                                hidmap[(c, ht)][:, ki * 512:(ki + 1) * 512],
                                start=(ht == 0), stop=(ht == HT - 1))
                    ot = opool.tile([128, 1024], F16, tag="ot")
                    if b2_zero:
                        nc.scalar.activation(ot[:], ps2[:], AF.Copy,
                                             bias=0.0, scale=1.0)
                    else:
                        nc.scalar.activation(ot[:], ps2[:], AF.Identity,
                                             bias=b2s[:, dt2:dt2 + 1],
                                             scale=1.0)
                    nc.sync.dma_start(
                        outT[dt2 * 128:(dt2 + 1) * 128,
                             2 * c * BC:(2 * c + 2) * BC], ot[:])

            # emb DMAs for the first two chunks go ahead of the term DMA so
            # their data lands while the carried L2 runs; the term tile is
            # filled by four k-quarter range DMAs so early evicts/adds wait
            # only on the quarter they actually read. The carried L2 runs
            # on the PE BEFORE the chunk-0/1 seg matmuls: it needs only
            # hid(7) (ready as the previous rep drains) and buys the
            # previous rep's chunk-7 evict time to free the pagg banks that
            # seg_mm(0) reuses (the recurring ~1.6us seam gap).
            pre0 = emit_emb_dma(0)
            pre1 = emit_emb_dma(1)
            tt = tpool.tile([128, T * DT * BC], F16, tag="term")
            TQ = T * DT * BC // 4
            for qi in range(4):
                nc.sync.dma_start(tt[:, qi * TQ:(qi + 1) * TQ],
                                  termT[:, qi * TQ:(qi + 1) * TQ])
            terms[0] = tt
            if carry["hids"] is not None:
                emit_l2(T // 2 - 1, carry["hids"])
                carry["hids"] = None
            emit_embmm(0, pre=pre0)
            emit_embmm(1, pre=pre1)
            for c in range(T // 2):
                k0, k1 = 2 * c, 2 * c + 1
                # both evicts first, always: evict(k1) must not queue behind
                # adds(k0) on the in-order DVE — it frees the pagg PSUM bank
                # that gates the chunk-(c+1) seg matmuls (and, at chunk 7,
                # the next rep's seg_mm(0)), and it unblocks a gpsimd
                # partner's adds early
                emit_evict(k0)
                emit_evict(k1)
                emit_adds(k0)
                emit_adds(k1)
                if c + 2 < T // 2:
                    emit_embmm(c + 2)
                if c > 0:
                    emit_l2(c - 1)
                emit_l1(c)
            if final:
                emit_l2(T // 2 - 1)
            else:
                carry["hids"] = hids

        if loop:
            ET = mybir.EngineType
            with tc.For_i(0, loop, 1,
                          hint_engines=(ET.PE, ET.DVE, ET.Activation, ET.SP)):
                body()
        else:
            for rep in range(repeats):
                body(final=(rep == repeats - 1))

    nc.compile()
    _KERNEL_CACHE[key] = nc
    return nc


def _prep_inputs(term_emb, pred_emb, inv_pred_emb, W1, b1, W2, b2, msgs):
    """Shard/transpose/cast host-side into the per-core device layouts."""
    import ml_dtypes
    f8 = ml_dtypes.float8_e4m3
    t16 = term_emb.astype(np.float16)
    emb = np.empty((NMSG, B, D), f8)
    for m, (_dst, _src, s, which, e) in enumerate(msgs):
        arr = pred_emb if which == 0 else inv_pred_emb
        if s == 1.0:
            emb[m] = arr[e].astype(f8)
        else:
            emb[m] = (s * arr[e]).astype(f8)
    w1_16 = np.ascontiguousarray(W1.astype(np.float16))
    w2_16 = np.ascontiguousarray(W2.astype(np.float16))
    b1t = np.ascontiguousarray(b1.astype(np.float32).reshape(HT, 128).T)
    b2t = np.ascontiguousarray(b2.astype(np.float32).reshape(DT, 128).T)
    ident = np.broadcast_to(np.eye(128, dtype=f8)[:, None, :],
                            (128, 2, 128))
    ident = np.ascontiguousarray(ident)
    in_maps = []
    for c in range(N_CORES):
        sl = slice(c * BC, (c + 1) * BC)
        termTc = np.ascontiguousarray(
            t16[:, sl, :].transpose(2, 0, 1).reshape(DT, 128, T, BC)
            .transpose(1, 2, 0, 3)).reshape(128, T * DT * BC)
        embTc = np.ascontiguousarray(
            emb[:, sl, :].transpose(2, 0, 1)).reshape(D, NMSG, BC)
        in_maps.append(dict(termT=termTc, embT=embTc, w1=w1_16, w2=w2_16,
                            b1t=b1t, b2t=b2t, ident=ident))
    return in_maps


def kernel(term_emb, pred_emb, inv_pred_emb, signs, W1, b1, W2, b2,
           heads, tails):
    term_emb = np.asarray(term_emb, dtype=np.float32)
    pred_emb = np.asarray(pred_emb, dtype=np.float32)
    inv_pred_emb = np.asarray(inv_pred_emb, dtype=np.float32)
    signs = np.asarray(signs, dtype=np.float32)
    W1 = np.asarray(W1, dtype=np.float32)
    b1 = np.asarray(b1, dtype=np.float32)
    W2 = np.asarray(W2, dtype=np.float32)
    b2 = np.asarray(b2, dtype=np.float32)
    heads = np.asarray(heads).astype(np.int64)
    tails = np.asarray(tails).astype(np.int64)

    msgs = _messages(heads, tails, signs)
    bias_zero = (not b1.any(), not b2.any())
    nc = _build(tuple(msgs), bias_zero=bias_zero)
    in_maps = _prep_inputs(term_emb, pred_emb, inv_pred_emb, W1, b1, W2, b2,
                           msgs)
    res = run_bass_kernel_spmd(nc, in_maps, list(range(N_CORES)))

    out = np.empty((T, B, D), np.float32)
    for c in range(N_CORES):
        o = res.results[c]["outT"].astype(np.float32)
        out[:, c * BC:(c + 1) * BC, :] = o.reshape(D, T, BC).transpose(1, 2, 0)
    return out
